# Initial kernel scaffold
#
"""Trainium2 Bass kernel for nn_DeformationCorrector.

Math (per particle, F = [[a,b],[c,d]], det F > 0 for this data):
  closed-form 2x2 SVD:  y1 = (a+d)^2 + (c-b)^2,  y2 = (a-d)^2 + (c+b)^2
    sq1 = sqrt(y1), sq2 = sqrt(y2);  sigma1 = (sq1+sq2)/2, sigma2 = (sq1-sq2)/2
  polar rotation R = U@Vh = [[p,-q],[q,p]],  p = (a+d)/sq1, q = (c-b)/sq1
  features (dedup; -1 shifts folded into b1):
    [sq1+sq2, sq1-sq2, a^2+c^2, ab+cd, b^2+d^2, ad-bc]  @ W1eff + b1eff
  MLP 6->128->128->3 (symmetrized W3), then delta = R @ x_sym, out = delta + F.

Distribution: pure data parallel over 8 cores, contiguous shards, weights
replicated. Layout conversions (particle-major elementwise <-> feature-major
matmul) go through cheap DRAM round trips instead of on-chip transposes.
"""

import os
from contextlib import ExitStack

import numpy as np

import concourse.bass as bass
import concourse.tile as tile
from concourse import mybir
from concourse.bass_utils import run_bass_kernel_spmd

NCORES = 8
P = 128
T = 512                 # matmul moving free dim (one PSUM bank of fp32)
CB = 512                # particles per partition per block
BLK = P * CB            # 65536 particles per block
NBLK = 2
NPC = NBLK * BLK        # 131072 particles per core (padded)
NTOT = NCORES * NPC     # 1048576
N = 1_000_000
HID = 128

CHUNKS_PER_BLK = BLK // T      # 128
GROUPS_PER_BLK = CHUNKS_PER_BLK // 4   # 32 (4 chunks per group: row/col packing)

FP32 = mybir.dt.float32
F32R = mybir.dt.float32r
AF = mybir.ActivationFunctionType
OP = mybir.AluOpType

# fraction of relu chunks handled by ACT (rest on DVE); x-drain alternates
RELU_ACT_OF10 = int(os.environ.get("K_RELU_ACT_OF10", "7"))

_built = {}
_last_results = None


def _relu_on_act(c):
    return (c % 10) < RELU_ACT_OF10


def build_program():
    nc = bass.Bass()

    F_in = nc.dram_tensor("F", [NPC, 4], FP32, kind="ExternalInput")
    W1S_in = nc.dram_tensor("W1S", [P, P], FP32, kind="ExternalInput")
    W2_in = nc.dram_tensor("W2", [P, P], FP32, kind="ExternalInput")
    W3S_in = nc.dram_tensor("W3S", [P, 4], FP32, kind="ExternalInput")
    B1_in = nc.dram_tensor("B1", [P, 1], FP32, kind="ExternalInput")
    B2_in = nc.dram_tensor("B2", [P, 1], FP32, kind="ExternalInput")
    B3S_in = nc.dram_tensor("B3S", [P, 1], FP32, kind="ExternalInput")
    OUT = nc.dram_tensor("OUT", [NPC, 4], FP32, kind="ExternalOutput")

    with ExitStack() as ctx, tile.TileContext(nc) as tc:
        consts = ctx.enter_context(tc.tile_pool(name="consts", bufs=1))
        fblk = ctx.enter_context(tc.tile_pool(name="fblk", bufs=NBLK))
        scr = ctx.enter_context(tc.tile_pool(name="scr", bufs=2))
        featp = ctx.enter_context(tc.tile_pool(name="featp", bufs=NBLK))
        dramp = ctx.enter_context(tc.tile_pool(name="dramp", bufs=NBLK, space="DRAM"))
        fmp = ctx.enter_context(tc.tile_pool(name="fmp", bufs=3))
        hp = ctx.enter_context(tc.tile_pool(name="hp", bufs=3))
        xp = ctx.enter_context(tc.tile_pool(name="xp", bufs=2))
        outp = ctx.enter_context(tc.tile_pool(name="outp", bufs=2))
        psz1 = ctx.enter_context(tc.tile_pool(name="psz1", bufs=2, space="PSUM"))
        psz2 = ctx.enter_context(tc.tile_pool(name="psz2", bufs=2, space="PSUM"))
        psx = ctx.enter_context(tc.tile_pool(name="psx", bufs=2, space="PSUM"))

        # ---- constants ----
        w1s_sb = consts.tile([P, P], FP32)
        nc.sync.dma_start(out=w1s_sb[:], in_=W1S_in[:, :])
        w2_sb = consts.tile([P, P], FP32)
        nc.sync.dma_start(out=w2_sb[:], in_=W2_in[:, :])
        w3s_sb = consts.tile([P, 4], FP32)
        nc.sync.dma_start(out=w3s_sb[:], in_=W3S_in[:, :])
        b1_sb = consts.tile([P, 1], FP32)
        nc.sync.dma_start(out=b1_sb[:], in_=B1_in[:, :])
        b2_sb = consts.tile([P, 1], FP32)
        nc.sync.dma_start(out=b2_sb[:], in_=B2_in[:, :])
        b3s_sb = consts.tile([P, 1], FP32)
        nc.sync.dma_start(out=b3s_sb[:], in_=B3S_in[:, :])

        f_tiles = []
        p_tiles = []
        q_tiles = []
        featd_tiles = []
        xd_tiles = []

        # ============ stage 1: particle-major features ============
        for b in range(NBLK):
            f_sb = fblk.tile([P, 4 * CB], FP32, tag="F", name=f"f_sb{b}")
            nc.sync.dma_start(
                out=f_sb[:],
                in_=F_in[:, :].rearrange("(b p c) k -> b p (c k)", b=NBLK, p=P)[b],
            )
            f_tiles.append(f_sb)
            fr = f_sb.rearrange("p (c k) -> p c k", k=4)
            av = fr[:, :, 0]
            bv = fr[:, :, 1]
            cv = fr[:, :, 2]
            dv = fr[:, :, 3]
            fr2 = f_sb.rearrange("p (c k2 k) -> p c k2 k", k2=2, k=2)
            ac = fr2[:, :, :, 0]   # (a, c) pairs
            bd = fr2[:, :, :, 1]   # (b, d) pairs

            feat_sb = featp.tile([P, 6 * CB], FP32, tag="feat", name=f"feat_sb{b}")
            fv = feat_sb.rearrange("p (f c) -> p f c", f=6)

            # squares on gpsimd: [aa, bb, cc, dd] interleaved
            sq_sb = scr.tile([P, 4 * CB], FP32, tag="sq", name=f"sq_sb{b}")
            nc.gpsimd.tensor_tensor(out=sq_sb[:], in0=f_sb[:], in1=f_sb[:], op=OP.mult)
            sqr = sq_sb.rearrange("p (c k) -> p c k", k=4)
            aa = sqr[:, :, 0]
            bb = sqr[:, :, 1]
            cc = sqr[:, :, 2]
            dd = sqr[:, :, 3]

            # (ab, cd) pairs on DVE
            pp_sb = scr.tile([P, 2 * CB], FP32, tag="pp", name=f"pp_sb{b}")
            ppv = pp_sb.rearrange("p (c k2) -> p c k2", k2=2)
            nc.vector.tensor_tensor(out=pp_sb[:], in0=ac, in1=bd, op=OP.mult)

            # f3 = ab + cd
            nc.vector.tensor_tensor(out=fv[:, 3], in0=ppv[:, :, 0], in1=ppv[:, :, 1], op=OP.add)
            # ad, bc, f5 = ad - bc  (gpsimd)
            ad_sb = scr.tile([P, CB], FP32, tag="ad", name=f"ad_sb{b}")
            nc.gpsimd.tensor_tensor(out=ad_sb[:], in0=av, in1=dv, op=OP.mult)
            bc_sb = scr.tile([P, CB], FP32, tag="bc", name=f"bc_sb{b}")
            nc.gpsimd.tensor_tensor(out=bc_sb[:], in0=bv, in1=cv, op=OP.mult)
            nc.gpsimd.tensor_tensor(out=fv[:, 5], in0=ad_sb[:], in1=bc_sb[:], op=OP.subtract)
            # f2 = aa + cc ; f4 = bb + dd
            nc.vector.tensor_tensor(out=fv[:, 2], in0=aa, in1=cc, op=OP.add)
            nc.vector.tensor_tensor(out=fv[:, 4], in0=bb, in1=dd, op=OP.add)
            # m = f2 + f4 ; y1 = 2*f5 + m ; y2 = max(-2*f5 + m, 0)
            m_sb = scr.tile([P, CB], FP32, tag="m", name=f"m_sb{b}")
            nc.vector.tensor_tensor(out=m_sb[:], in0=fv[:, 2], in1=fv[:, 4], op=OP.add)
            y1_sb = scr.tile([P, CB], FP32, tag="y1", name=f"y1_sb{b}")
            nc.vector.scalar_tensor_tensor(
                out=y1_sb[:], in0=fv[:, 5], scalar=2.0, in1=m_sb[:], op0=OP.mult, op1=OP.add
            )
            y2_sb = scr.tile([P, CB], FP32, tag="y2", name=f"y2_sb{b}")
            nc.vector.scalar_tensor_tensor(
                out=y2_sb[:], in0=fv[:, 5], scalar=-2.0, in1=m_sb[:], op0=OP.mult, op1=OP.add
            )
            nc.vector.tensor_scalar(
                out=y2_sb[:], in0=y2_sb[:], scalar1=0.0, scalar2=None, op0=OP.max
            )
            # sq1 = sqrt(y1), sq2 = sqrt(y2)  (ACT)
            sq1_sb = scr.tile([P, CB], FP32, tag="sq1", name=f"sq1_sb{b}")
            nc.scalar.activation(out=sq1_sb[:], in_=y1_sb[:], func=AF.Sqrt)
            sq2_sb = scr.tile([P, CB], FP32, tag="sq2", name=f"sq2_sb{b}")
            nc.scalar.activation(out=sq2_sb[:], in_=y2_sb[:], func=AF.Sqrt)
            # sigma features
            nc.vector.tensor_tensor(out=fv[:, 0], in0=sq1_sb[:], in1=sq2_sb[:], op=OP.add)
            nc.vector.tensor_tensor(out=fv[:, 1], in0=sq1_sb[:], in1=sq2_sb[:], op=OP.subtract)
            # s = a + d ; v = c - b ; rinv = 1/sq1 ; p = s*rinv ; q = v*rinv
            s_sb = scr.tile([P, CB], FP32, tag="s", name=f"s_sb{b}")
            nc.vector.tensor_tensor(out=s_sb[:], in0=av, in1=dv, op=OP.add)
            v_sb = scr.tile([P, CB], FP32, tag="v", name=f"v_sb{b}")
            nc.vector.tensor_tensor(out=v_sb[:], in0=cv, in1=bv, op=OP.subtract)
            rinv_sb = scr.tile([P, CB], FP32, tag="rinv", name=f"rinv_sb{b}")
            nc.vector.reciprocal_approx_fast(out=rinv_sb[:], in_=sq1_sb[:])
            p_sb = fblk.tile([P, CB], FP32, tag="p", name=f"p_sb{b}")
            nc.vector.tensor_tensor(out=p_sb[:], in0=s_sb[:], in1=rinv_sb[:], op=OP.mult)
            q_sb = fblk.tile([P, CB], FP32, tag="q", name=f"q_sb{b}")
            nc.vector.tensor_tensor(out=q_sb[:], in0=v_sb[:], in1=rinv_sb[:], op=OP.mult)
            p_tiles.append(p_sb)
            q_tiles.append(q_sb)

            # feat -> DRAM  [6, BLK], row f particle-ordered
            featd = dramp.tile([6, BLK], FP32, tag="featd", name=f"featd{b}")
            featd_tiles.append(featd)
            nc.sync.dma_start(
                out=featd[:, :].rearrange("f (p j) -> f p j", p=P).transpose([1, 0, 2]),
                in_=feat_sb.rearrange("p (f j) -> p f j", f=6),
            )

            xd = dramp.tile([3, BLK], FP32, tag="xd", name=f"xd{b}")
            xd_tiles.append(xd)

        # ============ stage 2: feature-major MLP ============
        cglobal = 0
        for b in range(NBLK):
            featd = featd_tiles[b]
            xd = xd_tiles[b]
            featd_cv = featd[:, :].rearrange("f (c j) -> f c j", j=T)
            xd_cv = xd[:, :].rearrange("k (c j) -> k c j", j=T)
            for i in range(GROUPS_PER_BLK):
                # one DMA pulls features of 4 chunks to partitions 32g+f
                featfm = fmp.tile([P, T], FP32, tag="featfm", name=f"ffm{b}_{i}")
                ffm_v = featfm.rearrange("(g r) j -> g r j", g=4)
                nc.sync.dma_start(
                    out=ffm_v[:, :6, :],
                    in_=featd_cv[:, 4 * i : 4 * i + 4, :].transpose([1, 0, 2]),
                )
                x_ps = psx.tile([P, T], FP32, tag="x", name=f"xps{b}_{i}")
                x_sb = xp.tile([P, T], FP32, tag="xsb", name=f"xsb{b}_{i}")
                for g in range(4):
                    z1 = psz1.tile([P, T], FP32, tag="z1", name=f"z1_{b}_{i}_{g}")
                    nc.tensor.matmul(
                        out=z1[:],
                        lhsT=w1s_sb[32 * g : 32 * g + 6, :].bitcast(F32R),
                        rhs=ffm_v[g, :6, :].bitcast(F32R),
                        tile_position=(32 * g, 0),
                    )
                    h1 = hp.tile([P, T], FP32, tag="h1", name=f"h1_{b}_{i}_{g}")
                    if _relu_on_act(cglobal):
                        nc.scalar.activation(out=h1[:], in_=z1[:], func=AF.Relu, bias=b1_sb[:])
                    else:
                        nc.vector.tensor_scalar(
                            out=h1[:], in0=z1[:], scalar1=b1_sb[:],
                            scalar2=0.0, op0=OP.add, op1=OP.max,
                        )
                    z2 = psz2.tile([P, T], FP32, tag="z2", name=f"z2_{b}_{i}_{g}")
                    nc.tensor.matmul(
                        out=z2[:], lhsT=w2_sb[:].bitcast(F32R), rhs=h1[:].bitcast(F32R)
                    )
                    h2 = hp.tile([P, T], FP32, tag="h2", name=f"h2_{b}_{i}_{g}")
                    if _relu_on_act(cglobal + 5):
                        nc.scalar.activation(out=h2[:], in_=z2[:], func=AF.Relu, bias=b2_sb[:])
                    else:
                        nc.vector.tensor_scalar(
                            out=h2[:], in0=z2[:], scalar1=b2_sb[:],
                            scalar2=0.0, op0=OP.add, op1=OP.max,
                        )
                    nc.tensor.matmul(
                        out=x_ps[32 * g : 32 * g + 3, :],
                        lhsT=w3s_sb[:, :3].bitcast(F32R),
                        rhs=h2[:].bitcast(F32R),
                        tile_position=(0, 32 * g),
                    )
                    cglobal += 1
                # drain x PSUM -> SBUF with +b3S, alternate engines
                if i % 2 == 0:
                    nc.scalar.activation(out=x_sb[:], in_=x_ps[:], func=AF.Identity, bias=b3s_sb[:])
                else:
                    nc.vector.tensor_scalar(
                        out=x_sb[:], in0=x_ps[:], scalar1=b3s_sb[:], scalar2=None, op0=OP.add
                    )
                # x -> DRAM rows k at chunk offsets
                xsb_v = x_sb.rearrange("(g r) j -> g r j", g=4)
                nc.sync.dma_start(
                    out=xd_cv[:, 4 * i : 4 * i + 4, :].transpose([1, 0, 2]),
                    in_=xsb_v[:, :3, :],
                )

        # ============ stage 3: particle-major backend ============
        for b in range(NBLK):
            xd = xd_tiles[b]
            f_sb = f_tiles[b]
            p_sb = p_tiles[b]
            q_sb = q_tiles[b]
            fr = f_sb.rearrange("p (c k) -> p c k", k=4)

            xs_all = xp.tile([P, 3 * CB], FP32, tag="xsall", name=f"xsall{b}")
            xs_v = xs_all.rearrange("p (k c) -> p k c", k=3)
            nc.sync.dma_start(
                out=xs_v,
                in_=xd[:, :].rearrange("k (p j) -> k p j", p=P).transpose([1, 0, 2]),
            )
            # Pall = p * [x0 x1 x2] ; Qall = q * [x0 x1 x2]   (gpsimd)
            pall = scr.tile([P, 3 * CB], FP32, tag="pall", name=f"pall{b}")
            pall_v = pall.rearrange("p (k c) -> p k c", k=3)
            nc.gpsimd.tensor_tensor(
                out=pall[:], in0=xs_all[:],
                in1=p_sb[:].unsqueeze(1).to_broadcast([P, 3, CB]), op=OP.mult,
            )
            qall = scr.tile([P, 3 * CB], FP32, tag="qall", name=f"qall{b}")
            qall_v = qall.rearrange("p (k c) -> p k c", k=3)
            nc.gpsimd.tensor_tensor(
                out=qall[:], in0=xs_all[:],
                in1=q_sb[:].unsqueeze(1).to_broadcast([P, 3, CB]), op=OP.mult,
            )
            out_sb = outp.tile([P, 4 * CB], FP32, tag="out", name=f"out_sb{b}")
            ov = out_sb.rearrange("p (c k) -> p c k", k=4)
            t0 = scr.tile([P, CB], FP32, tag="t0", name=f"t0_{b}")
            # d00 = P0 - Q1 + a
            nc.gpsimd.scalar_tensor_tensor(
                out=t0[:], in0=qall_v[:, 1], scalar=-1.0, in1=pall_v[:, 0],
                op0=OP.mult, op1=OP.add,
            )
            nc.gpsimd.tensor_tensor(out=ov[:, :, 0], in0=t0[:], in1=fr[:, :, 0], op=OP.add)
            # d01 = P1 - Q2 + b
            t1 = scr.tile([P, CB], FP32, tag="t1", name=f"t1_{b}")
            nc.gpsimd.scalar_tensor_tensor(
                out=t1[:], in0=qall_v[:, 2], scalar=-1.0, in1=pall_v[:, 1],
                op0=OP.mult, op1=OP.add,
            )
            nc.gpsimd.tensor_tensor(out=ov[:, :, 1], in0=t1[:], in1=fr[:, :, 1], op=OP.add)
            # d10 = Q0 + P1 + c
            t2 = scr.tile([P, CB], FP32, tag="t2", name=f"t2_{b}")
            nc.gpsimd.tensor_tensor(out=t2[:], in0=qall_v[:, 0], in1=pall_v[:, 1], op=OP.add)
            nc.gpsimd.tensor_tensor(out=ov[:, :, 2], in0=t2[:], in1=fr[:, :, 2], op=OP.add)
            # d11 = Q1 + P2 + d
            t3 = scr.tile([P, CB], FP32, tag="t3", name=f"t3_{b}")
            nc.gpsimd.tensor_tensor(out=t3[:], in0=qall_v[:, 1], in1=pall_v[:, 2], op=OP.add)
            nc.gpsimd.tensor_tensor(out=ov[:, :, 3], in0=t3[:], in1=fr[:, :, 3], op=OP.add)

            nc.sync.dma_start(
                out=OUT[:, :].rearrange("(b p c) k -> b p (c k)", b=NBLK, p=P)[b],
                in_=out_sb[:],
            )

    return nc


def prep_weights(W1, b1, W2, b2, W3, b3):
    """Host-side weight transforms (tiny)."""
    W1 = np.asarray(W1, np.float32)
    b1 = np.asarray(b1, np.float32)
    W2 = np.asarray(W2, np.float32)
    b2 = np.asarray(b2, np.float32)
    W3 = np.asarray(W3, np.float32)
    b3 = np.asarray(b3, np.float32)
    # features: [sq1+sq2, sq1-sq2, f2, f3, f4, f5]
    W1eff = np.stack(
        [0.5 * W1[0], 0.5 * W1[1], W1[2], W1[3] + W1[4], W1[5], W1[6]], axis=0
    )  # [6, 128]
    b1eff = b1 - (W1[0] + W1[1] + W1[2] + W1[5] + W1[6])
    W1S = np.zeros((P, P), np.float32)
    for g in range(4):
        W1S[32 * g : 32 * g + 6, :] = W1eff
    # symmetrized third layer: x_sym = [x00, (x01+x10)/2, x11]
    W3S = np.zeros((P, 4), np.float32)
    W3S[:, 0] = W3[:, 0]
    W3S[:, 1] = 0.5 * (W3[:, 1] + W3[:, 2])
    W3S[:, 2] = W3[:, 3]
    b3S3 = np.array([b3[0], 0.5 * (b3[1] + b3[2]), b3[3]], np.float32)
    B3S = np.zeros((P, 1), np.float32)
    for j in range(4):
        B3S[32 * j : 32 * j + 3, 0] = b3S3
    return {
        "W1S": W1S,
        "W2": np.ascontiguousarray(W2),
        "W3S": W3S,
        "B1": b1eff.reshape(P, 1).astype(np.float32),
        "B2": b2.reshape(P, 1).astype(np.float32),
        "B3S": B3S,
    }


def kernel(F, W1, b1, W2, b2, W3, b3):
    global _last_results
    F = np.asarray(F, np.float32).reshape(-1, 4)
    n = F.shape[0]
    assert n == N, f"expected {N} particles, got {n}"

    if "nc" not in _built:
        _built["nc"] = build_program()
    nc = _built["nc"]

    wmaps = prep_weights(W1, b1, W2, b2, W3, b3)
    Fpad = np.empty((NTOT, 4), np.float32)
    Fpad[:n] = F
    Fpad[n:] = np.array([1.0, 0.1, 0.0, 1.0], np.float32)

    in_maps = []
    for i in range(NCORES):
        m = {"F": np.ascontiguousarray(Fpad[i * NPC : (i + 1) * NPC])}
        m.update(wmaps)
        in_maps.append(m)

    res = run_bass_kernel_spmd(nc, in_maps, core_ids=list(range(NCORES)))
    _last_results = res
    out = np.concatenate([r["OUT"] for r in res.results], axis=0)[:n]
    return out.reshape(n, 2, 2).astype(np.float32)


# revision 15
# speedup vs baseline: 1.2829x; 1.2829x over previous
"""Trainium2 Bass kernel for nn_DeformationCorrector.

Math (per particle, F = [[a,b],[c,d]], det F > 0 for this data):
  closed-form 2x2 SVD:  y1 = (a+d)^2 + (c-b)^2,  y2 = (a-d)^2 + (c+b)^2
    sq1 = sqrt(y1), sq2 = sqrt(y2);  sigma1 = (sq1+sq2)/2, sigma2 = (sq1-sq2)/2
  polar rotation R = U@Vh = [[p,-q],[q,p]],  p = (a+d)/sq1, q = (c-b)/sq1
  features (dedup; -1 shifts folded into b1):
    [sq1+sq2, sq1-sq2, a^2+c^2, ab+cd, b^2+d^2, ad-bc]  @ W1eff + b1eff
  MLP 6->128->128->3 (symmetrized W3), then delta = R @ x_sym, out = delta + F.

Distribution: pure data parallel over 8 cores, contiguous shards, weights
replicated. Layout conversions (particle-major elementwise <-> feature-major
matmul) go through cheap DRAM round trips instead of on-chip transposes.
"""

import os
from contextlib import ExitStack

import numpy as np

import concourse.bass as bass
import concourse.bacc as bacc
import concourse.tile as tile
from concourse import mybir
from concourse.bass_utils import run_bass_kernel_spmd

NCORES = 8
P = 128
T = 512                 # matmul moving free dim (one PSUM bank of fp32)
CB = 512                # particles per partition per block
BLK = P * CB            # 65536 particles per block
NBLK = 2
NPC = NBLK * BLK        # 131072 particles per core (padded)
NTOT = NCORES * NPC     # 1048576
N = 1_000_000
HID = 128

CHUNKS_PER_BLK = BLK // T      # 128
GROUPS_PER_BLK = CHUNKS_PER_BLK // 4   # 32 (4 chunks per group: row/col packing)

FP32 = mybir.dt.float32
F32R = mybir.dt.float32r
BF16 = mybir.dt.bfloat16
AF = mybir.ActivationFunctionType
OP = mybir.AluOpType

# fraction of relu chunks handled by ACT (rest on DVE); x-drain alternates
RELU_ACT_OF10 = int(os.environ.get("K_RELU_ACT_OF10", "5"))

_built = {}
_last_results = None


def _relu_on_act(c):
    return (c % 10) < RELU_ACT_OF10


def build_program(nblk=NBLK, cb=CB, dbg=False):
    # local size overrides (for simulation/testing)
    global NBLK, CB, BLK, NPC, CHUNKS_PER_BLK, GROUPS_PER_BLK
    NBLK_s, CB_s = NBLK, CB
    NBLK, CB = nblk, cb
    BLK_l = P * CB
    NPC_l = NBLK * BLK_l
    try:
        nc = _build_impl(NBLK, CB, BLK_l, NPC_l, dbg)
    finally:
        NBLK, CB = NBLK_s, CB_s
    return nc


def _build_impl(NBLK, CB, BLK, NPC, dbg=False):
    CHUNKS_PER_BLK = BLK // T
    GROUPS_PER_BLK = CHUNKS_PER_BLK // 4
    nc = bacc.Bacc(trn_type="TRN2")

    F_in = nc.dram_tensor("F", [NPC, 4], FP32, kind="ExternalInput")
    W1S_in = nc.dram_tensor("W1S", [P, P], BF16, kind="ExternalInput")
    W2_in = nc.dram_tensor("W2", [P, P], BF16, kind="ExternalInput")
    W3S_in = nc.dram_tensor("W3S", [P, 32], BF16, kind="ExternalInput")
    B1_in = nc.dram_tensor("B1", [P, 1], FP32, kind="ExternalInput")
    B2_in = nc.dram_tensor("B2", [P, 1], FP32, kind="ExternalInput")
    B3S_in = nc.dram_tensor("B3S", [P, 1], FP32, kind="ExternalInput")
    OUT = nc.dram_tensor("OUT", [NPC, 4], FP32, kind="ExternalOutput")
    if dbg:
        FEATD = nc.dram_tensor("FEATD", [6, BLK], FP32, kind="ExternalOutput")
        XD = nc.dram_tensor("XD", [3, BLK], FP32, kind="ExternalOutput")
        PQ = nc.dram_tensor("PQ", [2 * P, CB], FP32, kind="ExternalOutput")
        H1D = nc.dram_tensor("H1D", [P, T], FP32, kind="ExternalOutput")

    with tile.TileContext(nc) as tc, ExitStack() as ctx:
        consts = ctx.enter_context(tc.tile_pool(name="consts", bufs=1))
        fblk = ctx.enter_context(tc.tile_pool(name="fblk", bufs=NBLK))
        scr = ctx.enter_context(tc.tile_pool(name="scr", bufs=2))
        featp = ctx.enter_context(tc.tile_pool(name="featp", bufs=NBLK))
        dramp = ctx.enter_context(tc.tile_pool(name="dramp", bufs=NBLK, space="DRAM"))
        fmp = ctx.enter_context(tc.tile_pool(name="fmp", bufs=6))
        hp = ctx.enter_context(tc.tile_pool(name="hp", bufs=6))
        xp = ctx.enter_context(tc.tile_pool(name="xp", bufs=2))
        outp = ctx.enter_context(tc.tile_pool(name="outp", bufs=2))
        psz1 = ctx.enter_context(tc.tile_pool(name="psz1", bufs=3, space="PSUM"))
        psz2 = ctx.enter_context(tc.tile_pool(name="psz2", bufs=3, space="PSUM"))
        psx = ctx.enter_context(tc.tile_pool(name="psx", bufs=2, space="PSUM"))

        # ---- constants ----
        w1s_sb = consts.tile([P, P], BF16)
        nc.sync.dma_start(out=w1s_sb[:], in_=W1S_in[:, :])
        w2_sb = consts.tile([P, P], BF16)
        nc.sync.dma_start(out=w2_sb[:], in_=W2_in[:, :])
        w3s_sb = consts.tile([P, 32], BF16)
        nc.sync.dma_start(out=w3s_sb[:], in_=W3S_in[:, :])
        b1_sb = consts.tile([P, 1], FP32)
        nc.sync.dma_start(out=b1_sb[:], in_=B1_in[:, :])
        b2_sb = consts.tile([P, 1], FP32)
        nc.sync.dma_start(out=b2_sb[:], in_=B2_in[:, :])
        b3s_sb = consts.tile([P, 1], FP32)
        nc.sync.dma_start(out=b3s_sb[:], in_=B3S_in[:, :])

        h1_dbg = []
        f_tiles = []
        p_tiles = []
        q_tiles = []
        featd_tiles = []
        xd_tiles = []

        # ============ stage 1: particle-major features ============
        for b in range(NBLK):
            f_sb = fblk.tile([P, 4 * CB], FP32, tag="F", name=f"f_sb{b}")
            nc.sync.dma_start(
                out=f_sb[:],
                in_=F_in[:, :].rearrange("(b p c) k -> b p (c k)", b=NBLK, p=P)[b],
            )
            f_tiles.append(f_sb)
            fr = f_sb.rearrange("p (c k) -> p c k", k=4)
            av = fr[:, :, 0]
            bv = fr[:, :, 1]
            cv = fr[:, :, 2]
            dv = fr[:, :, 3]
            fr2 = f_sb.rearrange("p (c k2 k) -> p c k2 k", k2=2, k=2)
            ac = fr2[:, :, :, 0]   # (a, c) pairs
            bd = fr2[:, :, :, 1]   # (b, d) pairs

            feat_sb = featp.tile([P, 6 * CB], FP32, tag="feat", name=f"feat_sb{b}")
            fv = feat_sb.rearrange("p (f c) -> p f c", f=6)

            # squares on gpsimd: [aa, bb, cc, dd] interleaved
            sq_sb = scr.tile([P, 4 * CB], FP32, tag="sq", name=f"sq_sb{b}")
            nc.gpsimd.tensor_tensor(out=sq_sb[:], in0=f_sb[:], in1=f_sb[:], op=OP.mult)
            sqr = sq_sb.rearrange("p (c k) -> p c k", k=4)
            aa = sqr[:, :, 0]
            bb = sqr[:, :, 1]
            cc = sqr[:, :, 2]
            dd = sqr[:, :, 3]

            # (ab, cd) pairs on DVE
            pp_sb = scr.tile([P, 2 * CB], FP32, tag="pp", name=f"pp_sb{b}")
            ppv = pp_sb.rearrange("p (c k2) -> p c k2", k2=2)
            nc.vector.tensor_tensor(out=pp_sb[:], in0=ac, in1=bd, op=OP.mult)

            # f3 = ab + cd
            nc.vector.tensor_tensor(out=fv[:, 3], in0=ppv[:, :, 0], in1=ppv[:, :, 1], op=OP.add)
            # ad, bc, f5 = ad - bc  (DVE so downstream STT has same-engine deps)
            ad_sb = scr.tile([P, CB], FP32, tag="ad", name=f"ad_sb{b}")
            nc.vector.tensor_tensor(out=ad_sb[:], in0=av, in1=dv, op=OP.mult)
            bc_sb = scr.tile([P, CB], FP32, tag="bc", name=f"bc_sb{b}")
            nc.gpsimd.tensor_tensor(out=bc_sb[:], in0=bv, in1=cv, op=OP.mult)
            nc.vector.tensor_tensor(out=fv[:, 5], in0=ad_sb[:], in1=bc_sb[:], op=OP.subtract)
            # f2 = aa + cc ; f4 = bb + dd
            nc.vector.tensor_tensor(out=fv[:, 2], in0=aa, in1=cc, op=OP.add)
            nc.vector.tensor_tensor(out=fv[:, 4], in0=bb, in1=dd, op=OP.add)
            # m = f2 + f4 ; y1 = 2*f5 + m ; y2 = max(-2*f5 + m, 0)
            m_sb = scr.tile([P, CB], FP32, tag="m", name=f"m_sb{b}")
            nc.vector.tensor_tensor(out=m_sb[:], in0=fv[:, 2], in1=fv[:, 4], op=OP.add)
            y1_sb = scr.tile([P, CB], FP32, tag="y1", name=f"y1_sb{b}")
            nc.vector.scalar_tensor_tensor(
                out=y1_sb[:], in0=fv[:, 5], scalar=2.0, in1=m_sb[:], op0=OP.mult, op1=OP.add
            )
            y2_sb = scr.tile([P, CB], FP32, tag="y2", name=f"y2_sb{b}")
            nc.vector.scalar_tensor_tensor(
                out=y2_sb[:], in0=fv[:, 5], scalar=-2.0, in1=m_sb[:], op0=OP.mult, op1=OP.add
            )
            nc.vector.tensor_scalar(
                out=y2_sb[:], in0=y2_sb[:], scalar1=0.0, scalar2=None, op0=OP.max
            )
            # sq1 = sqrt(y1), sq2 = sqrt(y2)  (ACT)
            sq1_sb = scr.tile([P, CB], FP32, tag="sq1", name=f"sq1_sb{b}")
            nc.scalar.activation(out=sq1_sb[:], in_=y1_sb[:], func=AF.Sqrt)
            sq2_sb = scr.tile([P, CB], FP32, tag="sq2", name=f"sq2_sb{b}")
            nc.scalar.activation(out=sq2_sb[:], in_=y2_sb[:], func=AF.Sqrt)
            # sigma features
            nc.vector.tensor_tensor(out=fv[:, 0], in0=sq1_sb[:], in1=sq2_sb[:], op=OP.add)
            nc.vector.tensor_tensor(out=fv[:, 1], in0=sq1_sb[:], in1=sq2_sb[:], op=OP.subtract)
            # s = a + d ; v = c - b ; rinv = 1/sq1 ; p = s*rinv ; q = v*rinv
            s_sb = scr.tile([P, CB], FP32, tag="s", name=f"s_sb{b}")
            nc.vector.tensor_tensor(out=s_sb[:], in0=av, in1=dv, op=OP.add)
            v_sb = scr.tile([P, CB], FP32, tag="v", name=f"v_sb{b}")
            nc.vector.tensor_tensor(out=v_sb[:], in0=cv, in1=bv, op=OP.subtract)
            rinv_sb = scr.tile([P, CB], FP32, tag="rinv", name=f"rinv_sb{b}")
            nc.vector.reciprocal_approx_fast(out=rinv_sb[:], in_=sq1_sb[:])
            p_sb = fblk.tile([P, CB], FP32, tag="p", name=f"p_sb{b}")
            nc.vector.tensor_tensor(out=p_sb[:], in0=s_sb[:], in1=rinv_sb[:], op=OP.mult)
            q_sb = fblk.tile([P, CB], FP32, tag="q", name=f"q_sb{b}")
            nc.vector.tensor_tensor(out=q_sb[:], in0=v_sb[:], in1=rinv_sb[:], op=OP.mult)
            p_tiles.append(p_sb)
            q_tiles.append(q_sb)

            # feat -> DRAM  [6, BLK], row f particle-ordered
            featd = dramp.tile([6, BLK], BF16, tag="featd", name=f"featd{b}")
            featd_tiles.append(featd)
            nc.gpsimd.dma_start(
                out=featd[:, :].rearrange("f (p j) -> f p j", p=P).transpose([1, 0, 2]),
                in_=feat_sb.rearrange("p (f j) -> p f j", f=6),
            )

            xd = dramp.tile([3, BLK], FP32, tag="xd", name=f"xd{b}")
            xd_tiles.append(xd)

        # ============ stage 2: feature-major MLP ============
        cglobal = 0
        for b in range(NBLK):
            featd = featd_tiles[b]
            xd = xd_tiles[b]
            featd_cv = featd[:, :].rearrange("f (c j) -> f c j", j=T)
            xd_cv = xd[:, :].rearrange("k (c j) -> k c j", j=T)
            for i in range(GROUPS_PER_BLK):
                # pull features of 4 chunks to partitions 32g+f (one DMA per chunk)
                featfm = fmp.tile([P, T], BF16, tag="featfm", name=f"ffm{b}_{i}")
                ffm_v = featfm.rearrange("(g r) j -> g r j", g=4)
                for g in range(4):
                    nc.sync.dma_start(
                        out=featfm[32 * g : 32 * g + 6, :],
                        in_=featd_cv[:, 4 * i + g, :],
                    )
                x_ps = psx.tile([P, T], FP32, tag="x", name=f"xps{b}_{i}")
                x_sb = xp.tile([P, T], FP32, tag="xsb", name=f"xsb{b}_{i}")
                z1s, h1s, z2s, h2s = [], [], [], []
                for g in range(4):
                    z1 = psz1.tile([P, T], FP32, tag="z1", name=f"z1_{b}_{i}_{g}")
                    nc.tensor.matmul(
                        out=z1[:],
                        lhsT=w1s_sb[32 * g : 32 * g + 6, :],
                        rhs=ffm_v[g, :6, :],
                        tile_position=(32 * g, 0),
                    )
                    z1s.append(z1)
                for g in range(4):
                    h1 = hp.tile([P, T], BF16, tag="h1", name=f"h1_{b}_{i}_{g}")
                    if _relu_on_act(cglobal + g):
                        nc.scalar.activation(out=h1[:], in_=z1s[g][:], func=AF.Relu, bias=b1_sb[:])
                    else:
                        nc.vector.tensor_scalar(
                            out=h1[:], in0=z1s[g][:], scalar1=b1_sb[:],
                            scalar2=0.0, op0=OP.add, op1=OP.max,
                        )
                    h1s.append(h1)
                if dbg and b == 0 and i == 0:
                    h1_dbg.append(h1s[0])
                for g in range(4):
                    z2 = psz2.tile([P, T], FP32, tag="z2", name=f"z2_{b}_{i}_{g}")
                    nc.tensor.matmul(out=z2[:], lhsT=w2_sb[:], rhs=h1s[g][:])
                    z2s.append(z2)
                for g in range(4):
                    h2 = hp.tile([P, T], BF16, tag="h2", name=f"h2_{b}_{i}_{g}")
                    if _relu_on_act(cglobal + g + 5):
                        nc.scalar.activation(out=h2[:], in_=z2s[g][:], func=AF.Relu, bias=b2_sb[:])
                    else:
                        nc.vector.tensor_scalar(
                            out=h2[:], in0=z2s[g][:], scalar1=b2_sb[:],
                            scalar2=0.0, op0=OP.add, op1=OP.max,
                        )
                    h2s.append(h2)
                for g in range(4):
                    nc.tensor.matmul(
                        out=x_ps[32 * g : 32 * g + 32, :],
                        lhsT=w3s_sb[:, :],
                        rhs=h2s[g][:],
                        tile_position=(0, 32 * g),
                    )
                cglobal += 4
                # drain x PSUM -> SBUF with +b3S, alternate engines
                if i % 2 == 0:
                    nc.scalar.activation(out=x_sb[:], in_=x_ps[:], func=AF.Identity, bias=b3s_sb[:])
                else:
                    nc.vector.tensor_scalar(
                        out=x_sb[:], in0=x_ps[:], scalar1=b3s_sb[:], scalar2=None, op0=OP.add
                    )
                # x -> DRAM rows k at chunk offsets (one DMA per chunk)
                for j in range(4):
                    nc.sync.dma_start(
                        out=xd_cv[:, 4 * i + j, :],
                        in_=x_sb[32 * j : 32 * j + 3, :],
                    )

        if dbg:
            nc.sync.dma_start(out=FEATD[:, :], in_=featd_tiles[0][:, :].bitcast(FP32))
            nc.sync.dma_start(out=XD[:, :], in_=xd_tiles[0][:, :])
            nc.sync.dma_start(out=PQ[:P, :], in_=p_tiles[0][:])
            nc.sync.dma_start(out=PQ[P:, :], in_=q_tiles[0][:])
            nc.sync.dma_start(out=H1D[:, :], in_=h1_dbg[0][:].bitcast(FP32))

        # ============ stage 3: particle-major backend ============
        for b in range(NBLK):
            xd = xd_tiles[b]
            f_sb = f_tiles[b]
            p_sb = p_tiles[b]
            q_sb = q_tiles[b]
            fr = f_sb.rearrange("p (c k) -> p c k", k=4)

            xs_all = xp.tile([P, 3 * CB], FP32, tag="xsall", name=f"xsall{b}")
            xs_v = xs_all.rearrange("p (k c) -> p k c", k=3)
            nc.sync.dma_start(
                out=xs_v,
                in_=xd[:, :].rearrange("k (p j) -> k p j", p=P).transpose([1, 0, 2]),
            )
            # Pall = p * [x0 x1 x2] ; Qall = q * [x0 x1 x2]   (gpsimd)
            pall = scr.tile([P, 3 * CB], FP32, tag="pall", name=f"pall{b}")
            pall_v = pall.rearrange("p (k c) -> p k c", k=3)
            nc.gpsimd.tensor_tensor(
                out=pall[:], in0=xs_all[:],
                in1=p_sb[:].unsqueeze(1).to_broadcast([P, 3, CB]), op=OP.mult,
            )
            qall = scr.tile([P, 3 * CB], FP32, tag="qall", name=f"qall{b}")
            qall_v = qall.rearrange("p (k c) -> p k c", k=3)
            nc.gpsimd.tensor_tensor(
                out=qall[:], in0=xs_all[:],
                in1=q_sb[:].unsqueeze(1).to_broadcast([P, 3, CB]), op=OP.mult,
            )
            out_sb = outp.tile([P, 4 * CB], FP32, tag="out", name=f"out_sb{b}")
            ov = out_sb.rearrange("p (c k) -> p c k", k=4)
            t0 = scr.tile([P, CB], FP32, tag="t0", name=f"t0_{b}")
            # d00 = P0 - Q1 + a
            nc.gpsimd.tensor_tensor(out=t0[:], in0=pall_v[:, 0], in1=qall_v[:, 1], op=OP.subtract)
            nc.gpsimd.tensor_tensor(out=ov[:, :, 0], in0=t0[:], in1=fr[:, :, 0], op=OP.add)
            # d01 = P1 - Q2 + b
            t1 = scr.tile([P, CB], FP32, tag="t1", name=f"t1_{b}")
            nc.gpsimd.tensor_tensor(out=t1[:], in0=pall_v[:, 1], in1=qall_v[:, 2], op=OP.subtract)
            nc.gpsimd.tensor_tensor(out=ov[:, :, 1], in0=t1[:], in1=fr[:, :, 1], op=OP.add)
            # d10 = Q0 + P1 + c
            t2 = scr.tile([P, CB], FP32, tag="t2", name=f"t2_{b}")
            nc.gpsimd.tensor_tensor(out=t2[:], in0=qall_v[:, 0], in1=pall_v[:, 1], op=OP.add)
            nc.gpsimd.tensor_tensor(out=ov[:, :, 2], in0=t2[:], in1=fr[:, :, 2], op=OP.add)
            # d11 = Q1 + P2 + d
            t3 = scr.tile([P, CB], FP32, tag="t3", name=f"t3_{b}")
            nc.gpsimd.tensor_tensor(out=t3[:], in0=qall_v[:, 1], in1=pall_v[:, 2], op=OP.add)
            nc.gpsimd.tensor_tensor(out=ov[:, :, 3], in0=t3[:], in1=fr[:, :, 3], op=OP.add)

            nc.sync.dma_start(
                out=OUT[:, :].rearrange("(b p c) k -> b p (c k)", b=NBLK, p=P)[b],
                in_=out_sb[:],
            )

    nc.finalize()
    return nc


def prep_weights(W1, b1, W2, b2, W3, b3):
    """Host-side weight transforms (tiny)."""
    W1 = np.asarray(W1, np.float32)
    b1 = np.asarray(b1, np.float32)
    W2 = np.asarray(W2, np.float32)
    b2 = np.asarray(b2, np.float32)
    W3 = np.asarray(W3, np.float32)
    b3 = np.asarray(b3, np.float32)
    # features: [sq1+sq2, sq1-sq2, f2, f3, f4, f5]
    W1eff = np.stack(
        [0.5 * W1[0], 0.5 * W1[1], W1[2], W1[3] + W1[4], W1[5], W1[6]], axis=0
    )  # [6, 128]
    b1eff = b1 - (W1[0] + W1[1] + W1[2] + W1[5] + W1[6])
    W1S = np.zeros((P, P), np.float32)
    for g in range(4):
        W1S[32 * g : 32 * g + 6, :] = W1eff
    # symmetrized third layer: x_sym = [x00, (x01+x10)/2, x11]
    W3S = np.zeros((P, 32), np.float32)
    W3S[:, 0] = W3[:, 0]
    W3S[:, 1] = 0.5 * (W3[:, 1] + W3[:, 2])
    W3S[:, 2] = W3[:, 3]
    b3S3 = np.array([b3[0], 0.5 * (b3[1] + b3[2]), b3[3]], np.float32)
    B3S = np.zeros((P, 1), np.float32)
    for j in range(4):
        B3S[32 * j : 32 * j + 3, 0] = b3S3
    import ml_dtypes
    return {
        "W1S": W1S.astype(ml_dtypes.bfloat16),
        "W2": W2.astype(ml_dtypes.bfloat16),
        "W3S": W3S.astype(ml_dtypes.bfloat16),
        "B1": b1eff.reshape(P, 1).astype(np.float32),
        "B2": b2.reshape(P, 1).astype(np.float32),
        "B3S": B3S,
    }


def kernel(F, W1, b1, W2, b2, W3, b3):
    global _last_results
    F = np.asarray(F, np.float32).reshape(-1, 4)
    n = F.shape[0]
    assert n == N, f"expected {N} particles, got {n}"

    if "nc" not in _built:
        _built["nc"] = build_program()
    nc = _built["nc"]

    wmaps = prep_weights(W1, b1, W2, b2, W3, b3)
    Fpad = np.empty((NTOT, 4), np.float32)
    Fpad[:n] = F
    Fpad[n:] = np.array([1.0, 0.1, 0.0, 1.0], np.float32)

    in_maps = []
    for i in range(NCORES):
        m = {"F": np.ascontiguousarray(Fpad[i * NPC : (i + 1) * NPC])}
        m.update(wmaps)
        in_maps.append(m)

    res = run_bass_kernel_spmd(nc, in_maps, core_ids=list(range(NCORES)))
    _last_results = res
    out = np.concatenate([r["OUT"] for r in res.results], axis=0)[:n]
    return out.reshape(n, 2, 2).astype(np.float32)
